# revision 1
# baseline (speedup 1.0000x reference)
"""GraphSAGE (2-layer, mean-agg) edge-scoring kernel for 8 trn2 NeuronCores.

  - Batch-parallel: core c handles edges [512c, 512(c+1)).
  - Projected tables sigmoid(feat @ W + b) in fp16 are built on-device into
    pair-shared HBM (cores 2k/2k+1 share one buffer; each projects half),
    synced with a pair AllReduce barrier.
  - Neighbor rows fetched with chunked dma_gather (int16 -> 25000-row
    chunks, <=1024 idx/call, 4 SWDGE queues), reordered/transposed to
    feat-major via SBUF-source transpose dma_gathers (hop-2 k-major so the
    10-way mean is 9 full-width vector adds; means folded into weights).
  - SAGE matmuls run feat-major (features on contraction partitions).
"""
import os
import numpy as np

F0 = F1 = 10
B = 4096
NCORES = 8
EDGES = B // NCORES          # 512
P = 128
D = 256
NU, NI = 50000, 100000
CHU = 25000                  # table chunk rows (int16-safe)
HALF_U, HALF_I = NU // 2, NI // 2
SEG_GROUPS = 640             # hop-2 groups per segment (= h1-token block)
NSEG = (EDGES * F0) // SEG_GROUPS    # 8
BLK = SEG_GROUPS
PROJ_TILE = 512


def _wrap16(a):
    a = np.asarray(a, np.int16)
    w = a.reshape(-1, 16).T
    return np.tile(w, (8, 1)).astype(np.int16)


def _pad128(n):
    return (n + 127) & ~127


class _HopPlan:
    def __init__(self, idx_lists, nrows, out_order, fixed_plen=None):
        self.nch = nrows // CHU
        M = len(idx_lists[0])
        runs = []
        for A in idx_lists:
            ch = A // CHU
            runs.append([np.where(ch == c)[0] for c in range(self.nch)])
        if fixed_plen is None:
            fixed_plen = [
                _pad128(max(len(r[c]) for r in runs)) for c in range(self.nch)]
        self.plen = fixed_plen
        self.runs = runs
        self.offs = np.concatenate([[0], np.cumsum(self.plen)]).astype(np.int64)
        self.tot = int(self.offs[-1])
        self.idx, self.rid = [], []
        for core, A in enumerate(idx_lists):
            iv = np.zeros(self.tot, np.int16)
            p2s = np.empty(M, np.int64)
            for c in range(self.nch):
                pos = runs[core][c]
                off = int(self.offs[c])
                iv[off:off + len(pos)] = (A[pos] - c * CHU).astype(np.int16)
                p2s[pos] = off + np.arange(len(pos))
            self.idx.append(iv)
            self.rid.append(p2s[out_order].astype(np.int16))
        self.calls = []
        for c in range(self.nch):
            off, rem = int(self.offs[c]), self.plen[c]
            while rem > 0:
                n = min(1024, rem)
                self.calls.append((c, off, n))
                off += n
                rem -= n


def _build_plans(inputs):
    plans = {}
    for side, (h0, h1, h2, t0, t1, t2) in {
        "s": (inputs["src_h0"], inputs["src_h1"], inputs["src_h2"], NU, NI, NU),
        "d": (inputs["dst_h0"], inputs["dst_h1"], inputs["dst_h2"], NI, NU, NI),
    }.items():
        h0 = np.asarray(h0).astype(np.int64).reshape(NCORES, EDGES)
        h1 = np.asarray(h1).astype(np.int64).reshape(NCORES, EDGES * F0)
        h2 = np.asarray(h2).astype(np.int64).reshape(NCORES, EDGES * F0 * F1)
        plans[side + "0"] = _HopPlan([h0[c] for c in range(NCORES)], t0,
                                     np.arange(EDGES))
        plans[side + "1"] = _HopPlan([h1[c] for c in range(NCORES)], t1,
                                     np.arange(EDGES * F0))
        # hop2 segments share one padded-run structure (max over cores+segs)
        oo = np.empty(SEG_GROUPS * F1, np.int64)
        for k in range(F1):
            oo[k * SEG_GROUPS:(k + 1) * SEG_GROUPS] = (
                np.arange(SEG_GROUPS) * F1 + k)
        nch = t2 // CHU
        seglists = [
            [h2[c][s * SEG_GROUPS * F1:(s + 1) * SEG_GROUPS * F1]
             for c in range(NCORES)] for s in range(NSEG)]
        plen = [0] * nch
        for s in range(NSEG):
            for A in seglists[s]:
                ch = A // CHU
                for c in range(nch):
                    plen[c] = max(plen[c], _pad128(int((ch == c).sum())))
        plans[side + "2"] = [
            _HopPlan(seglists[s], t2, oo, fixed_plen=plen) for s in range(NSEG)]
    return plans


def _proj_host(feat, half, ntiles):
    N = feat.shape[0]
    outs = []
    for parity in range(2):
        rows = np.arange(parity * half, (parity + 1) * half)
        padded = ntiles * PROJ_TILE
        rows_p = np.concatenate([rows, np.zeros(padded - half, np.int64)])
        order = rows_p.reshape(ntiles, P, 4).transpose(0, 2, 1).reshape(-1)
        # tile t, psum j, partition m -> original row order[t*512 + j*128 + m]
        xt = np.ascontiguousarray(feat[order].T.astype(np.float32))
        prow = np.empty((P, ntiles), np.int32)
        for t in range(ntiles):
            base = parity * half + t * PROJ_TILE
            pr = base + np.arange(P) * 4
            pr[pr >= (parity + 1) * half] = N
            prow[:, t] = pr // 4
        outs.append((xt, prow))
    return outs


def _build_bass(plans, ntu, nti, debug=False):
    import concourse.bass as bass
    import concourse.tile as tile
    import concourse.bacc as bacc
    from concourse import mybir, library_config
    from contextlib import ExitStack

    f16 = mybir.dt.float16
    f32 = mybir.dt.float32
    i16 = mybir.dt.int16
    i32 = mybir.dt.int32
    AF = mybir.ActivationFunctionType

    nc = bacc.Bacc("TRN2", target_bir_lowering=False, debug=False,
                   num_devices=NCORES, num_swdge_queues=4)

    xt_u = nc.dram_tensor("xt_u", [512, ntu * PROJ_TILE], f32, kind="ExternalInput")
    xt_i = nc.dram_tensor("xt_i", [512, nti * PROJ_TILE], f32, kind="ExternalInput")
    prow_u = nc.dram_tensor("prow_u", [P, ntu], i32, kind="ExternalInput")
    prow_i = nc.dram_tensor("prow_i", [P, nti], i32, kind="ExternalInput")
    w_pu = nc.dram_tensor("w_pu", [P, 4, D], f16, kind="ExternalInput")
    w_pi = nc.dram_tensor("w_pi", [P, 4, D], f16, kind="ExternalInput")
    b_p = nc.dram_tensor("b_p", [1, 2, D], f16, kind="ExternalInput")
    wsage = nc.dram_tensor("wsage", [P, 2, 2 * 768], f16, kind="ExternalInput")
    wlin = nc.dram_tensor("wlin", [P, 1], f16, kind="ExternalInput")
    blin = nc.dram_tensor("blin", [1, 1], f32, kind="ExternalInput")

    idx_t, rid_t = {}, {}
    for sd in ("s", "d"):
        p0, p1, seg2 = plans[sd + "0"], plans[sd + "1"], plans[sd + "2"]
        t2 = seg2[0].tot
        idx_t[sd + "0"] = nc.dram_tensor(f"idx{sd}0", [P, p0.tot // 16], i16,
                                         kind="ExternalInput")
        rid_t[sd + "0"] = nc.dram_tensor(f"rid{sd}0", [P, EDGES // 16], i16,
                                         kind="ExternalInput")
        idx_t[sd + "1"] = nc.dram_tensor(f"idx{sd}1", [P, p1.tot // 16], i16,
                                         kind="ExternalInput")
        rid_t[sd + "1"] = nc.dram_tensor(f"rid{sd}1", [P, EDGES * F0 // 16], i16,
                                         kind="ExternalInput")
        idx_t[sd + "2"] = nc.dram_tensor(f"idx{sd}2", [P, NSEG * t2 // 16], i16,
                                         kind="ExternalInput")
        rid_t[sd + "2"] = nc.dram_tensor(
            f"rid{sd}2", [P, NSEG * SEG_GROUPS * F1 // 16], i16,
            kind="ExternalInput")

    out = nc.dram_tensor("out", [1, EDGES], f32, kind="ExternalOutput")
    dbg = (nc.dram_tensor("dbg", [P, 2, EDGES], f32, kind="ExternalOutput")
           if debug else None)

    tab_u = nc.dram_tensor("tab_u", [NU + 4, D], f16, addr_space="Shared")
    tab_i = nc.dram_tensor("tab_i", [NI + 4, D], f16, addr_space="Shared")
    cc_in = nc.dram_tensor("cc_in", [1, 16], f32)
    cc_out = nc.dram_tensor("cc_out", [1, 16], f32)

    with tile.TileContext(nc) as tc, ExitStack() as ctx:
        nc.gpsimd.load_library(library_config.mlp)
        tc.strict_bb_all_engine_barrier()

        wpool = ctx.enter_context(tc.tile_pool(name="w", bufs=1))
        w_pu_s = wpool.tile([P, 4, D], f16, tag="wpu")
        w_pi_s = wpool.tile([P, 4, D], f16, tag="wpi")
        b_p_s = wpool.tile([1, 2, D], f16, tag="bp")
        wsage_s = wpool.tile([P, 2, 2 * 768], f16, tag="wsage")
        wlin_s = wpool.tile([P, 1], f16, tag="wlin")
        blin_s = wpool.tile([1, 1], f32, tag="blin")
        ones_s = wpool.tile([1, P], f16, tag="ones")
        for dst_, src_ in ((w_pu_s, w_pu), (w_pi_s, w_pi), (b_p_s, b_p),
                           (wsage_s, wsage), (wlin_s, wlin), (blin_s, blin)):
            nc.sync.dma_start(dst_[:], src_[:])
        nc.vector.memset(ones_s[:], 1.0)

        # ---------------- phase A: projection ----------------
        with tc.tile_pool(name="proj", bufs=3) as ppool, \
             tc.tile_pool(name="pps", bufs=2, space="PSUM") as pspool:

            def project(xt, prow, w_s, bcol, tab, ntiles):
                for t in range(ntiles):
                    xtt = ppool.tile([P, 4, PROJ_TILE], f16, tag="xtt")
                    nc.gpsimd.dma_start(
                        out=xtt[:],
                        in_=xt[:, t * PROJ_TILE:(t + 1) * PROJ_TILE].rearrange(
                            "(c p) n -> p c n", p=P))
                    prow_sb = ppool.tile([P, 1], i32, tag="prow")
                    nc.sync.dma_start(prow_sb[:], prow[:, t:t + 1])
                    sig = ppool.tile([P, 4, D], f16, tag="sig")
                    for j in range(4):
                        ps = pspool.tile([P, D], f32, tag="pps")
                        for c in range(4):
                            nc.tensor.matmul(
                                out=ps[:], lhsT=xtt[:, c, j * P:(j + 1) * P],
                                rhs=w_s[:, c, :], start=(c == 0), stop=False)
                        nc.tensor.matmul(out=ps[:], lhsT=ones_s[:, :],
                                         rhs=b_p_s[:, bcol, :], start=False,
                                         stop=True)
                        nc.scalar.activation(out=sig[:, j, :], in_=ps[:],
                                             func=AF.Sigmoid)
                    nc.gpsimd.indirect_dma_start(
                        out=tab[:, :].rearrange("(q r) d -> q (r d)", r=4),
                        out_offset=bass.IndirectOffsetOnAxis(
                            ap=prow_sb[:, :1], axis=0),
                        in_=sig[:].rearrange("p r d -> p (r d)"), in_offset=None)

            project(xt_u, prow_u, w_pu_s, 0, tab_u, ntu)
            project(xt_i, prow_i, w_pi_s, 1, tab_i, nti)

        # ---------------- phase B: pair barrier ----------------
        tc.strict_bb_all_engine_barrier()
        zz = wpool.tile([1, 16], f32, tag="zz")
        nc.vector.memset(zz[:], 1.0)
        nc.sync.dma_start(cc_in[:], zz[:])
        tc.strict_bb_all_engine_barrier()
        nc.gpsimd.collective_compute(
            "AllReduce", mybir.AluOpType.add,
            replica_groups=[[0, 1], [2, 3], [4, 5], [6, 7]],
            ins=[cc_in.ap()], outs=[cc_out.ap()])
        tc.strict_bb_all_engine_barrier()

        # ---------------- phase C: SAGE ----------------
        ipool = ctx.enter_context(tc.tile_pool(name="idx", bufs=1))
        i2pool = ctx.enter_context(tc.tile_pool(name="idx2", bufs=2))
        spool = ctx.enter_context(tc.tile_pool(name="stage", bufs=2))
        hpool = ctx.enter_context(tc.tile_pool(name="hts", bufs=1))
        kpool = ctx.enter_context(tc.tile_pool(name="kblk", bufs=2))
        vpool = ctx.enter_context(tc.tile_pool(name="vtmp", bufs=1))
        gpool = ctx.enter_context(tc.tile_pool(name="gts", bufs=1))
        ps2 = ctx.enter_context(tc.tile_pool(name="ps2", bufs=2, space="PSUM"))

        qn = [0]

        def gather_hbm(plan, idx_sb, coloff, tab, stage):
            for (c, off, n) in plan.calls:
                nc.gpsimd.dma_gather(
                    stage[:, off // P:(off + n) // P, :],
                    tab[c * CHU:(c + 1) * CHU, :],
                    idx_sb[:, coloff + off // 16: coloff + (off + n) // 16],
                    n, n, D, queue_num=qn[0] % 4)
                qn[0] += 1

        def regather(stage, rid_ap, n_out, dstT):
            nc.gpsimd.dma_gather(
                dstT[:], stage[:], rid_ap, n_out, n_out, D, transpose=True,
                sbuf_tokens_per_rank=P, sbuf_free_dim_per_rank=D * 2,
                queue_num=qn[0] % 4)
            qn[0] += 1

        def tree10_strided(src, dst, ngr):
            # src [P,2,ngr*10] fp16 (col j*10+k) -> dst [P,2,ngr] fp32
            t0_t = vpool.tile([P, 2, BLK], f32, tag="tr0")
            t0 = t0_t[:, :, :ngr]
            t1_t = vpool.tile([P, 2, BLK], f32, tag="tr1")
            t1 = t1_t[:, :, :ngr]
            v = src.rearrange("p c (j k) -> p c j k", k=F0)
            nc.vector.tensor_add(t0[:], v[:, :, :, 0], v[:, :, :, 1])
            for i in range(1, 5):
                nc.vector.tensor_add(t1[:], v[:, :, :, 2 * i], v[:, :, :, 2 * i + 1])
                if i < 4:
                    nc.vector.tensor_add(t0[:], t0[:], t1[:])
            nc.vector.tensor_add(dst, t0[:], t1[:])

        hts = {}
        for si, sd in enumerate(("s", "d")):
            p0, p1, seg2 = plans[sd + "0"], plans[sd + "1"], plans[sd + "2"]
            t2 = seg2[0].tot
            tA, tB = (tab_u, tab_i) if sd == "s" else (tab_i, tab_u)
            wof = si * 768
            ws0 = wsage_s[:, :, wof:wof + D]
            wa0 = wsage_s[:, :, wof + D:wof + 2 * D]
            ws1 = wsage_s[:, :, wof + 2 * D:wof + 2 * D + 128]
            wa1 = wsage_s[:, :, wof + 2 * D + 128:wof + 768]

            i0 = ipool.tile([P, p0.tot // 16], i16, tag="i0")
            nc.sync.dma_start(i0[:], idx_t[sd + "0"][:])
            r0 = ipool.tile([P, EDGES // 16], i16, tag="r0")
            nc.sync.dma_start(r0[:], rid_t[sd + "0"][:])
            i1 = ipool.tile([P, p1.tot // 16], i16, tag="i1")
            nc.sync.dma_start(i1[:], idx_t[sd + "1"][:])
            r1 = ipool.tile([P, EDGES * F0 // 16], i16, tag="r1")
            nc.sync.dma_start(r1[:], rid_t[sd + "1"][:])

            # --- h0 ---
            st0 = spool.tile([P, p0.tot // P, D], f16, tag="stg")
            gather_hbm(p0, i0, 0, tA, st0)
            h0T = hpool.tile([P, 2, EDGES], f16, tag="h0T")
            regather(st0, r0[:, :], EDGES, h0T)

            # --- h1 ---
            st1 = spool.tile([P, p1.tot // P, D], f16, tag="stg")
            gather_hbm(p1, i1, 0, tB, st1)
            h1T = hpool.tile([P, 2, EDGES * F0], f16, tag="h1T")
            n0f_t = vpool.tile([P, 2, EDGES], f32, tag="sumf")
            n0f = n0f_t[:, :, :EDGES]
            for b in range(EDGES * F0 // BLK):
                tmp = kpool.tile([P, 2, BLK], f16, tag="reT")
                regather(st1, r1[:, b * BLK // 16:(b + 1) * BLK // 16], BLK, tmp)
                nc.vector.tensor_copy(h1T[:, :, b * BLK:(b + 1) * BLK], tmp[:])
                tree10_strided(tmp[:], n0f[:, :, b * 64:(b + 1) * 64], 64)
            n0T = hpool.tile([P, 2, EDGES], f16, tag="n0T")
            nc.vector.tensor_copy(n0T[:], n0f[:])

            # --- h2 segments -> n1T ---
            n1T = hpool.tile([P, 2, EDGES * F0], f16, tag="n1T")
            for s in range(NSEG):
                pl = seg2[s]
                i2 = i2pool.tile([P, t2 // 16], i16, tag="i2")
                nc.sync.dma_start(i2[:], idx_t[sd + "2"][:, s * t2 // 16:
                                                         (s + 1) * t2 // 16])
                r2 = i2pool.tile([P, SEG_GROUPS * F1 // 16], i16, tag="r2")
                nc.sync.dma_start(
                    r2[:], rid_t[sd + "2"][:, s * SEG_GROUPS * F1 // 16:
                                           (s + 1) * SEG_GROUPS * F1 // 16])
                st2 = spool.tile([P, t2 // P, D], f16, tag="stg")
                gather_hbm(pl, i2, 0, tA, st2)
                t0 = vpool.tile([P, 2, BLK], f32, tag="tr0")
                t1 = vpool.tile([P, 2, BLK], f32, tag="tr1")
                ka = kpool.tile([P, 2, BLK], f16, tag="ka")
                kb = kpool.tile([P, 2, BLK], f16, tag="kb")
                for i in range(5):
                    regather(st2, r2[:, (2 * i) * BLK // 16:(2 * i + 1) * BLK // 16],
                             BLK, ka)
                    regather(st2, r2[:, (2 * i + 1) * BLK // 16:(2 * i + 2) * BLK // 16],
                             BLK, kb)
                    if i == 0:
                        nc.vector.tensor_add(t0[:], ka[:], kb[:])
                    else:
                        nc.vector.tensor_add(t1[:], ka[:], kb[:])
                        nc.vector.tensor_add(t0[:], t0[:], t1[:])
                nc.vector.tensor_copy(
                    n1T[:, :, s * SEG_GROUPS:(s + 1) * SEG_GROUPS], t0[:])

            # --- g1 = relu(h1 @ Ws0 + n1 @ Wa0) ---
            g1T = gpool.tile([P, 2, EDGES * F0], f16, tag="g1T")
            for o in range(2):
                for b in range(EDGES * F0 // BLK):
                    for half in range(2):
                        sl = slice(b * BLK + half * 320, b * BLK + (half + 1) * 320)
                        ps = ps2.tile([P, 320], f32, tag="g1ps")
                        for c in range(2):
                            nc.tensor.matmul(
                                out=ps[:], lhsT=ws0[:, c, o * P:(o + 1) * P],
                                rhs=h1T[:, c, sl], start=(c == 0), stop=False)
                            nc.tensor.matmul(
                                out=ps[:], lhsT=wa0[:, c, o * P:(o + 1) * P],
                                rhs=n1T[:, c, sl], start=False, stop=(c == 1))
                        nc.scalar.activation(out=g1T[:, o, sl], in_=ps[:],
                                             func=AF.Relu)

            nf_t = vpool.tile([P, 2, BLK], f32, tag="sumf")
            nf = nf_t[:, :, :EDGES]
            tree10_strided(g1T[:], nf, EDGES)
            nT = hpool.tile([P, 2, EDGES], f16, tag="nT")
            nc.vector.tensor_copy(nT[:], nf[:])

            # --- g0 = relu(h0 @ Ws0 + n0 @ Wa0) ---
            g0T = gpool.tile([P, 2, EDGES], f16, tag="g0T")
            for o in range(2):
                ps = ps2.tile([P, EDGES], f32, tag="mmps")
                for c in range(2):
                    nc.tensor.matmul(out=ps[:], lhsT=ws0[:, c, o * P:(o + 1) * P],
                                     rhs=h0T[:, c, :], start=(c == 0), stop=False)
                    nc.tensor.matmul(out=ps[:], lhsT=wa0[:, c, o * P:(o + 1) * P],
                                     rhs=n0T[:, c, :], start=False, stop=(c == 1))
                nc.scalar.activation(out=g0T[:, o, :], in_=ps[:], func=AF.Relu)

            # --- hT = g0 @ Ws1 + n @ Wa1 ---
            ps = ps2.tile([P, EDGES], f32, tag="mmps")
            for c in range(2):
                nc.tensor.matmul(out=ps[:], lhsT=ws1[:, c, :], rhs=g0T[:, c, :],
                                 start=(c == 0), stop=False)
                nc.tensor.matmul(out=ps[:], lhsT=wa1[:, c, :], rhs=nT[:, c, :],
                                 start=False, stop=(c == 1))
            hT = gpool.tile([P, EDGES], f16, tag=f"hT{sd}")
            nc.scalar.activation(out=hT[:], in_=ps[:], func=AF.Copy)
            hts[sd] = hT

        prod = gpool.tile([P, EDGES], f16, tag="prod")
        nc.vector.tensor_mul(prod[:], hts["s"][:], hts["d"][:])
        psf = ps2.tile([1, EDGES], f32, tag="fps")
        nc.tensor.matmul(out=psf[:], lhsT=wlin_s[:], rhs=prod[:],
                         start=True, stop=True)
        res = gpool.tile([1, EDGES], f32, tag="res")
        nc.scalar.activation(out=res[:], in_=psf[:], func=AF.Identity,
                             bias=blin_s[:, :1])
        nc.sync.dma_start(out[:], res[:])
        if debug:
            dv = gpool.tile([P, 2, EDGES], f32, tag="dv")
            nc.vector.tensor_copy(dv[:, 0, :], hts["s"][:])
            nc.vector.tensor_copy(dv[:, 1, :], hts["d"][:])
            nc.sync.dma_start(dbg[:], dv[:])

    nc.compile()
    return nc


def kernel(**inputs) -> np.ndarray:
    from concourse.bass_utils import run_bass_kernel_spmd

    plans = _build_plans(inputs)
    ntu = -(-HALF_U // PROJ_TILE)   # 49
    nti = -(-HALF_I // PROJ_TILE)   # 98

    trace = bool(os.environ.get("GNN_TRACE"))
    debug = bool(os.environ.get("GNN_DEBUG"))
    if trace:
        import timing_shim
        timing_shim.install()

    nc = _build_bass(plans, ntu, nti, debug=debug)

    uf = np.asarray(inputs["user_feat"], np.float32)
    itf = np.asarray(inputs["item_feat"], np.float32)
    proj_u = _proj_host(uf, HALF_U, ntu)
    proj_i = _proj_host(itf, HALF_I, nti)

    f16 = np.float16
    w_pu = np.ascontiguousarray(
        np.asarray(inputs["W_pu"], np.float32).reshape(4, P, D)
        .transpose(1, 0, 2)).astype(f16)
    w_pi = np.ascontiguousarray(
        np.asarray(inputs["W_pi"], np.float32).reshape(4, P, D)
        .transpose(1, 0, 2)).astype(f16)
    b_p = np.stack([np.asarray(inputs["b_pu"], np.float32),
                    np.asarray(inputs["b_pi"], np.float32)])[None].astype(f16)

    def sagew(pre):
        s0 = np.asarray(inputs[f"{pre}_self0"], np.float32)
        a0 = np.asarray(inputs[f"{pre}_agg0"], np.float32) * (1.0 / F0)
        s1 = np.asarray(inputs[f"{pre}_self1"], np.float32)
        a1 = np.asarray(inputs[f"{pre}_agg1"], np.float32) * (1.0 / F0)
        cat = np.concatenate([s0, a0, s1, a1], axis=1)  # [256, 768]
        return cat.reshape(2, P, 768).transpose(1, 0, 2)

    wsage = np.ascontiguousarray(
        np.concatenate([sagew("u"), sagew("i")], axis=2)).astype(f16)
    wlin = np.asarray(inputs["W_lin"], np.float32).astype(f16)
    blin = np.asarray(inputs["b_lin"], np.float32).reshape(1, 1)

    in_maps = []
    for c in range(NCORES):
        par = c % 2
        m = {
            "xt_u": proj_u[par][0], "prow_u": proj_u[par][1],
            "xt_i": proj_i[par][0], "prow_i": proj_i[par][1],
            "w_pu": w_pu, "w_pi": w_pi, "b_p": b_p,
            "wsage": wsage, "wlin": wlin, "blin": blin,
        }
        for sd in ("s", "d"):
            p0, p1, seg2 = plans[sd + "0"], plans[sd + "1"], plans[sd + "2"]
            m[f"idx{sd}0"] = _wrap16(p0.idx[c])
            m[f"rid{sd}0"] = _wrap16(p0.rid[c])
            m[f"idx{sd}1"] = _wrap16(p1.idx[c])
            m[f"rid{sd}1"] = _wrap16(p1.rid[c])
            m[f"idx{sd}2"] = np.concatenate(
                [_wrap16(pl.idx[c]) for pl in seg2], axis=1)
            m[f"rid{sd}2"] = np.concatenate(
                [_wrap16(pl.rid[c]) for pl in seg2], axis=1)
        in_maps.append(m)

    kw = dict(trace=True, trace_cores=list(range(NCORES))) if trace else {}
    res = run_bass_kernel_spmd(nc, in_maps, core_ids=list(range(NCORES)), **kw)
    if trace and res.exec_time_ns:
        print(f"HW exec time: {res.exec_time_ns} ns")
        kernel.last_exec_ns = res.exec_time_ns
    if debug:
        kernel.last_dbg = [res.results[c]["dbg"] for c in range(NCORES)]

    logits = np.concatenate([res.results[c]["out"][0] for c in range(NCORES)])
    return logits.reshape(B, 1).astype(np.float32)



# revision 11
# speedup vs baseline: 1.6559x; 1.6559x over previous
"""GraphSAGE (2-layer, mean-agg) edge-scoring kernel for 8 trn2 NeuronCores.

Per-core private compacted tables (no pair sharing, no barrier):
  - h0/h1 hop tables stored in token order (plain / transposing DMA loads).
  - hop-2 rows deduped into 2 tables per side (segs {0,1} and {2,3,4}),
    each guaranteed <= 30720 rows so plain positive int16 dma_gather idx work
    with no chunking and no reorder (gathers are order-preserving,
    1024-idx calls, k-major so the 10-way mean is 9 strided vector adds).
  - Projection reads host-prepacked fp16 feature tiles (HWDGE), writes each
    table slot contiguously; per-table dram tensors let Tile overlap SAGE
    gathers with the tail of projection.
  - SAGE matmuls run feat-major; token-major gathered data is transposed
    via TensorE (identity matmul) in packed PSUM groups.
"""
import os
import numpy as np

F0 = F1 = 10
B = 4096
NCORES = 8
EDGES = B // NCORES          # 512
P = 128
D = 256
NU, NI = 50000, 100000
SEG = 1024                   # hop-2 groups per segment
NSEG = (EDGES * F0) // SEG   # 5
NSEG_A = 2                   # segs 0,1 -> table A; segs 2.. -> table B
CALL = 1024                  # dma_gather idx per call (HW cap)
PROJ_TILE = 512


def _wrap16(a):
    a = np.asarray(a, np.int16)
    w = a.reshape(-1, 16).T
    return np.tile(w, (8, 1)).astype(np.int16)


def _pad512(n):
    return max(512, (int(n) + 511) & ~511)


def _build_plans(inputs):
    """Per-core compacted tables + k-major hop-2 gather indices.

    Returns dict with:
      sizes: dict slot -> padded row count (max over cores)
      rows:  per core: dict slot -> original-table row ids (np.int64 array)
      idx2:  per core: dict side -> [NSEG*F1*CALL] int16 gather idx (k-major)
    """
    h = {}
    for k in ("src_h0", "src_h1", "src_h2", "dst_h0", "dst_h1", "dst_h2"):
        h[k] = np.asarray(inputs[k]).astype(np.int64).reshape(NCORES, -1)

    rows = [dict() for _ in range(NCORES)]
    idx2 = [dict() for _ in range(NCORES)]
    for c in range(NCORES):
        for sd in ("s", "d"):
            pre = "src" if sd == "s" else "dst"
            h0 = h[pre + "_h0"][c]
            h1 = h[pre + "_h1"][c]
            h2 = h[pre + "_h2"][c]
            rows[c][sd + "h0"] = h0
            rows[c][sd + "h1"] = h1
            a_tok = h2[: NSEG_A * SEG * F1]
            b_tok = h2[NSEG_A * SEG * F1:]
            ua, inva = np.unique(a_tok, return_inverse=True)
            ub, invb = np.unique(b_tok, return_inverse=True)
            assert len(ua) <= 32000 and len(ub) <= 32000
            rows[c][sd + "A"] = ua
            rows[c][sd + "B"] = ub
            calls = []
            for s in range(NSEG):
                inv = inva if s < NSEG_A else invb
                off = 0 if s < NSEG_A else NSEG_A * SEG * F1
                base = s * SEG * F1 - off
                for k in range(F1):
                    # call position q (=token within seg) -> table pos
                    calls.append(inv[base + np.arange(SEG) * F1 + k])
            idx2[c][sd] = np.concatenate(calls)
    sizes = {}
    for slot in ("sh0", "sh1", "sA", "sB", "dh0", "dh1", "dA", "dB"):
        sizes[slot] = _pad512(max(len(rows[c][slot]) for c in range(NCORES)))
    return {"sizes": sizes, "rows": rows, "idx2": idx2}


# slot -> (feature kind, bias column) ; src side: h0/h2=user, h1=item
_SLOT_KIND = {"sh0": "u", "sh1": "i", "sA": "u", "sB": "u",
              "dh0": "i", "dh1": "u", "dA": "i", "dB": "i"}
_SLOTS = ("sh0", "dh0", "sh1", "dh1", "sA", "dA", "sB", "dB")


def _proj_host(feat_u16, feat_i16, plan):
    """Build per-core xt: [512, TOT] fp16, feature-major, tile-permuted.

    Within each 512-col tile, col q = cdim*128 + m holds row m*4 + cdim of
    the slot (so the projected PSUM groups store as contiguous row spans).
    """
    sizes = plan["sizes"]
    xts = []
    for c in range(NCORES):
        parts = []
        for slot in _SLOTS:
            r = plan["rows"][c][slot]
            n = sizes[slot]
            cols = np.zeros((n,), np.int64)
            cols[:len(r)] = r
            perm = cols.reshape(n // PROJ_TILE, P, 4).transpose(
                0, 2, 1).reshape(-1)
            feat = feat_u16 if _SLOT_KIND[slot] == "u" else feat_i16
            parts.append(feat[:, perm])
        xts.append(np.ascontiguousarray(np.concatenate(parts, axis=1)))
    return xts


def _build_bass(plan, debug=False):
    import concourse.tile as tile
    import concourse.bacc as bacc
    from concourse import mybir, library_config
    from concourse.masks import make_identity
    from contextlib import ExitStack

    f16 = mybir.dt.float16
    f32 = mybir.dt.float32
    i16 = mybir.dt.int16
    AF = mybir.ActivationFunctionType

    sizes = plan["sizes"]
    tot = sum(sizes[s] for s in _SLOTS)

    nc = bacc.Bacc("TRN2", target_bir_lowering=False, debug=False,
                   num_devices=NCORES, num_swdge_queues=4)

    xt = nc.dram_tensor("xt", [512, tot], f16, kind="ExternalInput")
    w_pu = nc.dram_tensor("w_pu", [P, 4, D], f16, kind="ExternalInput")
    w_pi = nc.dram_tensor("w_pi", [P, 4, D], f16, kind="ExternalInput")
    b_p = nc.dram_tensor("b_p", [1, 2, D], f16, kind="ExternalInput")
    wsage = nc.dram_tensor("wsage", [P, 2, 2 * 768], f16, kind="ExternalInput")
    wlin = nc.dram_tensor("wlin", [P, 1], f16, kind="ExternalInput")
    blin = nc.dram_tensor("blin", [1, 1], f32, kind="ExternalInput")
    idx_s = nc.dram_tensor("idx_s", [P, NSEG * F1 * CALL // 16], i16,
                           kind="ExternalInput")
    idx_d = nc.dram_tensor("idx_d", [P, NSEG * F1 * CALL // 16], i16,
                           kind="ExternalInput")
    out = nc.dram_tensor("out", [1, EDGES], f32, kind="ExternalOutput")

    tabs = {s: nc.dram_tensor(f"tab_{s}", [sizes[s], D], f16)
            for s in _SLOTS}

    with tile.TileContext(nc) as tc, ExitStack() as ctx:
        nc.gpsimd.load_library(library_config.mlp)
        tc.strict_bb_all_engine_barrier()

        wpool = ctx.enter_context(tc.tile_pool(name="w", bufs=1))
        w_pu_s = wpool.tile([P, 4, D], f16, tag="wpu")
        w_pi_s = wpool.tile([P, 4, D], f16, tag="wpi")
        b_p_s = wpool.tile([1, 2, D], f16, tag="bp")
        wsage_s = wpool.tile([P, 2, 2 * 768], f16, tag="wsage")
        wlin_s = wpool.tile([P, 1], f16, tag="wlin")
        blin_s = wpool.tile([1, 1], f32, tag="blin")
        ones_s = wpool.tile([1, P], f16, tag="ones")
        ident = wpool.tile([P, P], f16, tag="ident")
        for dst_, src_ in ((w_pu_s, w_pu), (w_pi_s, w_pi), (b_p_s, b_p),
                           (wsage_s, wsage), (wlin_s, wlin), (blin_s, blin)):
            nc.sync.dma_start(dst_[:], src_[:])
        nc.vector.memset(ones_s[:], 1.0)
        make_identity(nc, ident[:])

        # ---------------- phase A: projection ----------------
        ppool = ctx.enter_context(tc.tile_pool(name="proj", bufs=2))
        pspool = ctx.enter_context(tc.tile_pool(name="pps", bufs=2,
                                                space="PSUM"))
        col_off = 0
        for slot in _SLOTS:
            tab = tabs[slot]
            kind = _SLOT_KIND[slot]
            w_s = w_pu_s if kind == "u" else w_pi_s
            bcol = 0 if kind == "u" else 1
            ntiles = sizes[slot] // PROJ_TILE
            for t in range(ntiles):
                base = col_off + t * PROJ_TILE
                xtt = ppool.tile([P, 4, PROJ_TILE], f16, tag="xtt")
                nc.sync.dma_start(
                    xtt[:],
                    xt[:, base:base + PROJ_TILE].rearrange(
                        "(c p) n -> p c n", p=P))
                sig = ppool.tile([P, 4, D], f16, tag="sig")
                for j in range(4):
                    ps = pspool.tile([P, D], f32, tag="pps")
                    for cch in range(4):
                        nc.tensor.matmul(
                            out=ps[:], lhsT=xtt[:, cch, j * P:(j + 1) * P],
                            rhs=w_s[:, cch, :], start=(cch == 0), stop=False)
                    nc.tensor.matmul(out=ps[:], lhsT=ones_s[:, :],
                                     rhs=b_p_s[:, bcol, :], start=False,
                                     stop=True)
                    nc.scalar.activation(out=sig[:, j, :], in_=ps[:],
                                         func=AF.Sigmoid)
                nc.scalar.dma_start(
                    tab[t * PROJ_TILE:(t + 1) * PROJ_TILE, :].rearrange(
                        "(p r) d -> p (r d)", r=4),
                    sig[:].rearrange("p r d -> p (r d)"))
            col_off += sizes[slot]

        # ---------------- phase C: SAGE ----------------
        hpool = ctx.enter_context(tc.tile_pool(name="hts", bufs=1))
        # shared scratch: [P,8,256] f32 is the largest shape under this tag
        vpool = ctx.enter_context(tc.tile_pool(name="vtmp", bufs=3))
        plpool = ctx.enter_context(tc.tile_pool(name="pl", bufs=4))
        acpool = ctx.enter_context(tc.tile_pool(name="ac", bufs=2))
        i2pool = ctx.enter_context(tc.tile_pool(name="i2p", bufs=2))
        gpool = ctx.enter_context(tc.tile_pool(name="gts", bufs=1))
        ps2 = ctx.enter_context(tc.tile_pool(name="ps2", bufs=2, space="PSUM"))
        pst = ctx.enter_context(tc.tile_pool(name="pst", bufs=2, space="PSUM"))

        qn = [0]

        def tree10(src, dst, ngr):
            # src [P,2,ngr*10] f16 (col g*10+k) -> dst [P,2,ngr] f32 sums
            t0_t = vpool.tile([P, 2, EDGES], f32, tag="tr")
            t0 = t0_t[:, :, :ngr]
            t1_t = vpool.tile([P, 2, EDGES], f32, tag="tr")
            t1 = t1_t[:, :, :ngr]
            v = src.rearrange("p c (j k) -> p c j k", k=F0)
            nc.vector.tensor_add(t0[:], v[:, :, :, 0], v[:, :, :, 1])
            for i in range(1, 5):
                nc.vector.tensor_add(t1[:], v[:, :, :, 2 * i],
                                     v[:, :, :, 2 * i + 1])
                if i < 4:
                    nc.vector.tensor_add(t0[:], t0[:], t1[:])
            nc.vector.tensor_add(dst, t0[:], t1[:])

        hts = {}
        for si, sd in enumerate(("s", "d")):
            wof = si * 768
            ws0 = wsage_s[:, :, wof:wof + D]
            wa0 = wsage_s[:, :, wof + D:wof + 2 * D]
            ws1 = wsage_s[:, :, wof + 2 * D:wof + 2 * D + 128]
            wa1 = wsage_s[:, :, wof + 2 * D + 128:wof + 768]
            tab_h0 = tabs[sd + "h0"]
            tab_h1 = tabs[sd + "h1"]

            # --- h0T / h1T via transposing DMA loads (token order tables) ---
            h0T = hpool.tile([P, 2, EDGES], f16, tag="h0T")
            h1T = hpool.tile([P, 2, EDGES * F0], f16, tag="h1T")
            for f in range(2):
                nc.sync.dma_start_transpose(
                    h0T[:, f, :], tab_h0[:, f * P:(f + 1) * P])
                nc.sync.dma_start_transpose(
                    h1T[:, f, :], tab_h1[:, f * P:(f + 1) * P])

            # --- n0T = group-sums of h1T ---
            n0f = vpool.tile([P, 2, EDGES], f32, tag="tr")
            tree10(h1T[:], n0f[:], EDGES)
            n0T = hpool.tile([P, 2, EDGES], f16, tag="n0T")
            nc.vector.tensor_copy(n0T[:], n0f[:])

            # --- hop-2 segments -> n1T (rotating k-planes, chained accum) ---
            i2t = i2pool.tile([P, NSEG * F1 * CALL // 16], i16, tag="i2t")
            nc.sync.dma_start(i2t[:], (idx_s if sd == "s" else idx_d)[:])
            n1T = hpool.tile([P, 2, EDGES * F0], f16, tag="n1T")
            for s in range(NSEG):
                tabAB = tabs[sd + ("A" if s < NSEG_A else "B")]
                acc = acpool.tile([P, SEG // P, D], f32, tag="acc")
                planes = []
                for k in range(F1):
                    pl = plpool.tile([P, SEG // P, D], f16, tag="pl")
                    co = (s * F1 + k) * (CALL // 16)
                    nc.gpsimd.dma_gather(
                        pl[:], tabAB[:, :], i2t[:, co:co + CALL // 16],
                        CALL, CALL, D, queue_num=qn[0] % 4)
                    qn[0] += 1
                    planes.append(pl)
                    if k == 1:
                        nc.vector.tensor_add(acc[:], planes[0][:], planes[1][:])
                    elif k > 1:
                        nc.vector.tensor_add(acc[:], acc[:], pl[:])
                n1s = vpool.tile([P, SEG // P, D], f16, tag="n1s")
                nc.vector.tensor_copy(n1s[:], acc[:])
                # transpose to feat-major: 16 tiles packed 4-per-psum
                for f in range(2):
                    for g in range(2):
                        pt = pst.tile([P, 4, P], f16, tag="pt")
                        for b in range(4):
                            nc.tensor.matmul(
                                out=pt[:, b, :],
                                lhsT=n1s[:, g * 4 + b, f * P:(f + 1) * P],
                                rhs=ident[:], is_transpose=True)
                        dst = n1T[:, f, s * SEG + g * 512:
                                  s * SEG + (g + 1) * 512]
                        if (f + g) % 2 == 0:
                            nc.scalar.activation(out=dst, in_=pt[:].rearrange(
                                "p b q -> p (b q)"), func=AF.Copy)
                        else:
                            nc.vector.tensor_copy(dst, pt[:].rearrange(
                                "p b q -> p (b q)"))

            # --- g1 = relu(h1 @ Ws0 + n1 @ Wa0) ---
            g1T = gpool.tile([P, 2, EDGES * F0], f16, tag="g1T")
            for o in range(2):
                for bb in range(EDGES * F0 // 512):
                    sl = slice(bb * 512, (bb + 1) * 512)
                    ps = ps2.tile([P, 512], f32, tag="mmps")
                    for cch in range(2):
                        nc.tensor.matmul(
                            out=ps[:], lhsT=ws0[:, cch, o * P:(o + 1) * P],
                            rhs=h1T[:, cch, sl], start=(cch == 0), stop=False)
                        nc.tensor.matmul(
                            out=ps[:], lhsT=wa0[:, cch, o * P:(o + 1) * P],
                            rhs=n1T[:, cch, sl], start=False, stop=(cch == 1))
                    nc.scalar.activation(out=g1T[:, o, sl], in_=ps[:],
                                         func=AF.Relu)

            # --- nT = group-sums of g1T ---
            nf = vpool.tile([P, 2, EDGES], f32, tag="tr")
            tree10(g1T[:], nf[:], EDGES)
            nT = hpool.tile([P, 2, EDGES], f16, tag="nT")
            nc.vector.tensor_copy(nT[:], nf[:])

            # --- g0 = relu(h0 @ Ws0 + n0 @ Wa0) ---
            g0T = gpool.tile([P, 2, EDGES], f16, tag="g0T")
            for o in range(2):
                ps = ps2.tile([P, 512], f32, tag="mmps")
                for cch in range(2):
                    nc.tensor.matmul(out=ps[:],
                                     lhsT=ws0[:, cch, o * P:(o + 1) * P],
                                     rhs=h0T[:, cch, :], start=(cch == 0),
                                     stop=False)
                    nc.tensor.matmul(out=ps[:],
                                     lhsT=wa0[:, cch, o * P:(o + 1) * P],
                                     rhs=n0T[:, cch, :], start=False,
                                     stop=(cch == 1))
                nc.scalar.activation(out=g0T[:, o, :], in_=ps[:], func=AF.Relu)

            # --- hT = g0 @ Ws1 + n @ Wa1 ---
            ps = ps2.tile([P, 512], f32, tag="mmps")
            for cch in range(2):
                nc.tensor.matmul(out=ps[:], lhsT=ws1[:, cch, :],
                                 rhs=g0T[:, cch, :], start=(cch == 0),
                                 stop=False)
                nc.tensor.matmul(out=ps[:], lhsT=wa1[:, cch, :],
                                 rhs=nT[:, cch, :], start=False,
                                 stop=(cch == 1))
            hT = gpool.tile([P, EDGES], f16, tag=f"hT{sd}")
            nc.scalar.activation(out=hT[:], in_=ps[:], func=AF.Copy)
            hts[sd] = hT

        prod = gpool.tile([P, EDGES], f16, tag="prod")
        nc.vector.tensor_mul(prod[:], hts["s"][:], hts["d"][:])
        psf = pspool.tile([1, EDGES], f32, tag="fps")
        nc.tensor.matmul(out=psf[:], lhsT=wlin_s[:], rhs=prod[:],
                         start=True, stop=True)
        res = gpool.tile([1, EDGES], f32, tag="res")
        nc.scalar.activation(out=res[:], in_=psf[:], func=AF.Identity,
                             bias=blin_s[:, :1])
        nc.sync.dma_start(out[:], res[:])

    nc.compile()
    return nc


def kernel(**inputs) -> np.ndarray:
    from concourse.bass_utils import run_bass_kernel_spmd

    plan = _build_plans(inputs)

    trace = bool(os.environ.get("GNN_TRACE"))
    if trace:
        import timing_shim
        timing_shim.install()

    nc = _build_bass(plan)

    f16 = np.float16
    # feature tables, feature-major fp16 [512, N]
    feat_u16 = np.ascontiguousarray(
        np.asarray(inputs["user_feat"], np.float32).T).astype(f16)
    feat_i16 = np.ascontiguousarray(
        np.asarray(inputs["item_feat"], np.float32).T).astype(f16)
    xts = _proj_host(feat_u16, feat_i16, plan)

    w_pu = np.ascontiguousarray(
        np.asarray(inputs["W_pu"], np.float32).reshape(4, P, D)
        .transpose(1, 0, 2)).astype(f16)
    w_pi = np.ascontiguousarray(
        np.asarray(inputs["W_pi"], np.float32).reshape(4, P, D)
        .transpose(1, 0, 2)).astype(f16)
    b_p = np.stack([np.asarray(inputs["b_pu"], np.float32),
                    np.asarray(inputs["b_pi"], np.float32)])[None].astype(f16)

    def sagew(pre):
        s0 = np.asarray(inputs[f"{pre}_self0"], np.float32)
        a0 = np.asarray(inputs[f"{pre}_agg0"], np.float32) * (1.0 / F0)
        s1 = np.asarray(inputs[f"{pre}_self1"], np.float32)
        a1 = np.asarray(inputs[f"{pre}_agg1"], np.float32) * (1.0 / F0)
        cat = np.concatenate([s0, a0, s1, a1], axis=1)  # [256, 768]
        return cat.reshape(2, P, 768).transpose(1, 0, 2)

    wsage = np.ascontiguousarray(
        np.concatenate([sagew("u"), sagew("i")], axis=2)).astype(f16)
    wlin = np.asarray(inputs["W_lin"], np.float32).astype(f16)
    blin = np.asarray(inputs["b_lin"], np.float32).reshape(1, 1)

    in_maps = []
    for c in range(NCORES):
        m = {
            "xt": xts[c], "w_pu": w_pu, "w_pi": w_pi, "b_p": b_p,
            "wsage": wsage, "wlin": wlin, "blin": blin,
            "idx_s": _wrap16(plan["idx2"][c]["s"]),
            "idx_d": _wrap16(plan["idx2"][c]["d"]),
        }
        in_maps.append(m)

    kw = dict(trace=True, trace_cores=list(range(NCORES))) if trace else {}
    res = run_bass_kernel_spmd(nc, in_maps, core_ids=list(range(NCORES)), **kw)
    if trace and res.exec_time_ns:
        print(f"HW exec time: {res.exec_time_ns} ns")
        kernel.last_exec_ns = res.exec_time_ns

    logits = np.concatenate([res.results[c]["out"][0] for c in range(NCORES)])
    return logits.reshape(B, 1).astype(np.float32)


# revision 18
# speedup vs baseline: 1.9332x; 1.1675x over previous
"""GraphSAGE (2-layer, mean-agg) edge-scoring kernel for 8 trn2 NeuronCores.

Per-core private compacted tables (no pair sharing, no barrier):
  - h0/h1 hop tables stored in token order (plain / transposing DMA loads).
  - hop-2 rows deduped into 2 tables per side (segs {0,1} and {2,3,4}),
    each guaranteed <= 30720 rows so plain positive int16 dma_gather idx work
    with no chunking and no reorder (gathers are order-preserving,
    1024-idx calls, k-major so the 10-way mean is 9 strided vector adds).
  - Projection reads host-prepacked fp16 feature tiles (HWDGE), writes each
    table slot contiguously; per-table dram tensors let Tile overlap SAGE
    gathers with the tail of projection.
  - SAGE matmuls run feat-major; token-major gathered data is transposed
    via TensorE (identity matmul) in packed PSUM groups.
"""
import os
import numpy as np

F0 = F1 = 10
B = 4096
NCORES = 8
EDGES = B // NCORES          # 512
P = 128
D = 256
NU, NI = 50000, 100000
SEG = 1024                   # hop-2 groups per segment
NSEG = (EDGES * F0) // SEG   # 5
NSEG_A = 2                   # segs 0,1 -> table A; segs 2.. -> table B
CALL = 1024                  # dma_gather idx per call (HW cap)
PROJ_TILE = 512


def _wrap16(a):
    a = np.asarray(a, np.int16)
    w = a.reshape(-1, 16).T
    return np.tile(w, (8, 1)).astype(np.int16)


def _pad512(n):
    return max(512, (int(n) + 511) & ~511)


def _build_plans(inputs):
    """Per-core compacted tables + k-major hop-2 gather indices.

    Returns dict with:
      sizes: dict slot -> padded row count (max over cores)
      rows:  per core: dict slot -> original-table row ids (np.int64 array)
      idx2:  per core: dict side -> [NSEG*F1*CALL] int16 gather idx (k-major)
    """
    h = {}
    for k in ("src_h0", "src_h1", "src_h2", "dst_h0", "dst_h1", "dst_h2"):
        h[k] = np.asarray(inputs[k]).astype(np.int64).reshape(NCORES, -1)

    rows = [dict() for _ in range(NCORES)]
    idx2 = [dict() for _ in range(NCORES)]
    for c in range(NCORES):
        for sd in ("s", "d"):
            pre = "src" if sd == "s" else "dst"
            h0 = h[pre + "_h0"][c]
            h1 = h[pre + "_h1"][c]
            h2 = h[pre + "_h2"][c]
            rows[c][sd + "h0"] = h0
            rows[c][sd + "h1"] = h1
            a_tok = h2[: NSEG_A * SEG * F1]
            b_tok = h2[NSEG_A * SEG * F1:]
            ua, inva = np.unique(a_tok, return_inverse=True)
            ub, invb = np.unique(b_tok, return_inverse=True)
            assert len(ua) <= 32000 and len(ub) <= 32000
            rows[c][sd + "A"] = ua
            rows[c][sd + "B"] = ub
            calls = []
            for s in range(NSEG):
                inv = inva if s < NSEG_A else invb
                off = 0 if s < NSEG_A else NSEG_A * SEG * F1
                base = s * SEG * F1 - off
                for k in range(F1):
                    # call position q (=token within seg) -> table pos
                    calls.append(inv[base + np.arange(SEG) * F1 + k])
            idx2[c][sd] = np.concatenate(calls)
    sizes = {}
    for slot in ("sh0", "sh1", "sA", "sB", "dh0", "dh1", "dA", "dB"):
        sizes[slot] = _pad512(max(len(rows[c][slot]) for c in range(NCORES)))
    return {"sizes": sizes, "rows": rows, "idx2": idx2}


# slot -> (feature kind, bias column) ; src side: h0/h2=user, h1=item
_SLOT_KIND = {"sh0": "u", "sh1": "i", "sA": "u", "sB": "u",
              "dh0": "i", "dh1": "u", "dA": "i", "dB": "i"}
_SLOTS = ("sh0", "dh0", "sh1", "dh1", "sA", "dA", "sB", "dB")


def _proj_host(feat_u16, feat_i16, plan):
    """Build per-core xt: [512, TOT] fp16, feature-major, tile-permuted.

    Within each 512-col tile, col q = cdim*128 + m holds row m*4 + cdim of
    the slot (so the projected PSUM groups store as contiguous row spans).
    """
    sizes = plan["sizes"]
    xts = []
    for c in range(NCORES):
        parts = []
        for slot in _SLOTS:
            r = plan["rows"][c][slot]
            n = sizes[slot]
            cols = np.zeros((n,), np.int64)
            cols[:len(r)] = r
            perm = cols.reshape(n // PROJ_TILE, P, 4).transpose(
                0, 2, 1).reshape(-1)
            feat = feat_u16 if _SLOT_KIND[slot] == "u" else feat_i16
            parts.append(feat[:, perm])
        xts.append(np.ascontiguousarray(np.concatenate(parts, axis=1)))
    return xts


def _build_bass(plan, debug=False):
    import concourse.tile as tile
    import concourse.bacc as bacc
    from concourse import mybir, library_config
    from concourse.masks import make_identity
    from contextlib import ExitStack

    f16 = mybir.dt.float16
    f32 = mybir.dt.float32
    i16 = mybir.dt.int16
    AF = mybir.ActivationFunctionType

    sizes = plan["sizes"]
    tot = sum(sizes[s] for s in _SLOTS)

    nc = bacc.Bacc("TRN2", target_bir_lowering=False, debug=False,
                   num_devices=NCORES, num_swdge_queues=4)

    xt = nc.dram_tensor("xt", [512, tot], f16, kind="ExternalInput")
    w_pu = nc.dram_tensor("w_pu", [P, 4, D], f16, kind="ExternalInput")
    w_pi = nc.dram_tensor("w_pi", [P, 4, D], f16, kind="ExternalInput")
    b_p = nc.dram_tensor("b_p", [1, 2, 2 * D], f16, kind="ExternalInput")
    wsage = nc.dram_tensor("wsage", [P, 2, 2 * 768], f16, kind="ExternalInput")
    wlin = nc.dram_tensor("wlin", [P, 1], f16, kind="ExternalInput")
    blin = nc.dram_tensor("blin", [1, 1], f32, kind="ExternalInput")
    idx_s = nc.dram_tensor("idx_s", [P, NSEG * F1 * CALL // 16], i16,
                           kind="ExternalInput")
    idx_d = nc.dram_tensor("idx_d", [P, NSEG * F1 * CALL // 16], i16,
                           kind="ExternalInput")
    out = nc.dram_tensor("out", [1, EDGES], f32, kind="ExternalOutput")

    tabs = {s: nc.dram_tensor(f"tab_{s}", [sizes[s], D], f16)
            for s in _SLOTS}

    with tile.TileContext(nc) as tc, ExitStack() as ctx:
        nc.gpsimd.load_library(library_config.mlp)
        tc.strict_bb_all_engine_barrier()

        wpool = ctx.enter_context(tc.tile_pool(name="w", bufs=1))
        w_pu_s = wpool.tile([P, 4, D], f16, tag="wpu")
        w_pi_s = wpool.tile([P, 4, D], f16, tag="wpi")
        b_p_s = wpool.tile([1, 2, 2 * D], f16, tag="bp")
        wsage_s = wpool.tile([P, 2, 2 * 768], f16, tag="wsage")
        wlin_s = wpool.tile([P, 1], f16, tag="wlin")
        blin_s = wpool.tile([1, 1], f32, tag="blin")
        ones_s = wpool.tile([1, P], f16, tag="ones")
        ident = wpool.tile([P, P], f16, tag="ident")
        for dst_, src_ in ((w_pu_s, w_pu), (w_pi_s, w_pi), (b_p_s, b_p),
                           (wsage_s, wsage), (wlin_s, wlin), (blin_s, blin)):
            nc.sync.dma_start(dst_[:], src_[:])
        nc.vector.memset(ones_s[:], 1.0)
        make_identity(nc, ident[:])

        # ---------------- phase A: projection ----------------
        ppool = ctx.enter_context(tc.tile_pool(name="proj", bufs=2))
        pspool = ctx.enter_context(tc.tile_pool(name="pps", bufs=2,
                                                space="PSUM"))
        col_off = 0
        for slot in _SLOTS:
            tab = tabs[slot]
            kind = _SLOT_KIND[slot]
            w_s = w_pu_s if kind == "u" else w_pi_s
            bcol = 0 if kind == "u" else 1
            ntiles = sizes[slot] // PROJ_TILE
            for t in range(ntiles):
                base = col_off + t * PROJ_TILE
                xtt = ppool.tile([P, 4, PROJ_TILE], f16, tag="xtt")
                nc.sync.dma_start(
                    xtt[:],
                    xt[:, base:base + PROJ_TILE].rearrange(
                        "(c p) n -> p c n", p=P))
                sig = ppool.tile([P, 4, D], f16, tag="sig")
                sgl = sig[:].rearrange("p a b -> p (a b)")
                # one PSUM bank per 2 j-groups; bias matmul opens the
                # accumulation group over the whole bank, mains accumulate
                for hb in range(2):
                    ps = pspool.tile([P, 2, D], f32, tag="pps")
                    psl = ps[:].rearrange("p a b -> p (a b)")
                    nc.tensor.matmul(out=psl[:, :], lhsT=ones_s[:, :],
                                     rhs=b_p_s[:, bcol, :], start=True,
                                     stop=False)
                    for jj in range(2):
                        j = 2 * hb + jj
                        for cch in range(4):
                            nc.tensor.matmul(
                                out=ps[:, jj, :],
                                lhsT=xtt[:, cch, j * P:(j + 1) * P],
                                rhs=w_s[:, cch, :], start=False,
                                stop=(cch == 3))
                    nc.scalar.activation(out=sgl[:, hb * 512:(hb + 1) * 512],
                                         in_=psl[:, :], func=AF.Sigmoid)
                nc.scalar.dma_start(
                    tab[t * PROJ_TILE:(t + 1) * PROJ_TILE, :].rearrange(
                        "(p r) d -> p (r d)", r=4),
                    sig[:].rearrange("p r d -> p (r d)"))
            col_off += sizes[slot]

        # ---------------- phase C: SAGE ----------------
        hpool = ctx.enter_context(tc.tile_pool(name="hts", bufs=1))
        # shared scratch: [P,8,256] f32 is the largest shape under this tag
        vpool = ctx.enter_context(tc.tile_pool(name="vtmp", bufs=3))
        plpool = ctx.enter_context(tc.tile_pool(name="pl", bufs=4))
        acpool = ctx.enter_context(tc.tile_pool(name="ac", bufs=2))
        i2pool = ctx.enter_context(tc.tile_pool(name="i2p", bufs=2))
        gpool = ctx.enter_context(tc.tile_pool(name="gts", bufs=1))
        ps2 = ctx.enter_context(tc.tile_pool(name="ps2", bufs=2, space="PSUM"))
        pst = ctx.enter_context(tc.tile_pool(name="pst", bufs=2, space="PSUM"))

        qn = [0]

        def tree10(src, dst, ngr):
            # src [P,2,ngr*10] f16 (col g*10+k) -> dst [P,2,ngr] f32 sums
            t0_t = vpool.tile([P, 2, EDGES], f32, tag="tr")
            t0 = t0_t[:, :, :ngr]
            t1_t = vpool.tile([P, 2, EDGES], f32, tag="tr")
            t1 = t1_t[:, :, :ngr]
            v = src.rearrange("p c (j k) -> p c j k", k=F0)
            nc.vector.tensor_add(t0[:], v[:, :, :, 0], v[:, :, :, 1])
            for i in range(1, 5):
                nc.vector.tensor_add(t1[:], v[:, :, :, 2 * i],
                                     v[:, :, :, 2 * i + 1])
                if i < 4:
                    nc.vector.tensor_add(t0[:], t0[:], t1[:])
            nc.vector.tensor_add(dst, t0[:], t1[:])

        hts = {}
        for si, sd in enumerate(("s", "d")):
            wof = si * 768
            ws0 = wsage_s[:, :, wof:wof + D]
            wa0 = wsage_s[:, :, wof + D:wof + 2 * D]
            ws1 = wsage_s[:, :, wof + 2 * D:wof + 2 * D + 128]
            wa1 = wsage_s[:, :, wof + 2 * D + 128:wof + 768]
            tab_h0 = tabs[sd + "h0"]
            tab_h1 = tabs[sd + "h1"]

            # --- h0T / h1T via transposing DMA loads (token order tables) ---
            h0T = hpool.tile([P, 2, EDGES], f16, tag="h0T")
            h1T = hpool.tile([P, 2, EDGES * F0], f16, tag="h1T")
            for f in range(2):
                nc.sync.dma_start_transpose(
                    h0T[:, f, :], tab_h0[:, f * P:(f + 1) * P])
                nc.sync.dma_start_transpose(
                    h1T[:, f, :], tab_h1[:, f * P:(f + 1) * P])

            # --- n0T = group-sums of h1T ---
            n0f = vpool.tile([P, 2, EDGES], f32, tag="tr")
            tree10(h1T[:], n0f[:], EDGES)
            n0T = hpool.tile([P, 2, EDGES], f16, tag="n0T")
            nc.vector.tensor_copy(n0T[:], n0f[:])

            # --- hop-2 segments -> n1T (rotating k-planes, chained accum) ---
            i2t = i2pool.tile([P, NSEG * F1 * CALL // 16], i16, tag="i2t")
            nc.sync.dma_start(i2t[:], (idx_s if sd == "s" else idx_d)[:])
            n1T = hpool.tile([P, 2, EDGES * F0], f16, tag="n1T")
            for s in range(NSEG):
                tabAB = tabs[sd + ("A" if s < NSEG_A else "B")]
                acc = acpool.tile([P, SEG // P, D], f16, tag="acc")
                planes = []
                for k in range(F1):
                    pl = plpool.tile([P, SEG // P, D], f16, tag="pl")
                    co = (s * F1 + k) * (CALL // 16)
                    nc.gpsimd.dma_gather(
                        pl[:], tabAB[:, :], i2t[:, co:co + CALL // 16],
                        CALL, CALL, D, queue_num=qn[0] % 4)
                    qn[0] += 1
                    planes.append(pl)
                    if k == 1:
                        nc.vector.tensor_add(acc[:], planes[0][:], planes[1][:])
                    elif k > 1:
                        nc.vector.tensor_add(acc[:], acc[:], pl[:])
                n1s = acc
                # transpose to feat-major: 16 tiles packed 4-per-psum
                for f in range(2):
                    for g in range(2):
                        pt = pst.tile([P, 4, P], f16, tag="pt")
                        for b in range(4):
                            nc.tensor.matmul(
                                out=pt[:, b, :],
                                lhsT=n1s[:, g * 4 + b, f * P:(f + 1) * P],
                                rhs=ident[:], is_transpose=True)
                        dst = n1T[:, f, s * SEG + g * 512:
                                  s * SEG + (g + 1) * 512]
                        if (f + g) % 2 == 0:
                            nc.scalar.activation(out=dst, in_=pt[:].rearrange(
                                "p b q -> p (b q)"), func=AF.Copy)
                        else:
                            nc.vector.tensor_copy(dst, pt[:].rearrange(
                                "p b q -> p (b q)"))

            # --- g1 = relu(h1 @ Ws0 + n1 @ Wa0) ---
            g1T = gpool.tile([P, 2, EDGES * F0], f16, tag="g1T")
            for o in range(2):
                for bb in range(EDGES * F0 // 512):
                    sl = slice(bb * 512, (bb + 1) * 512)
                    ps = ps2.tile([P, 512], f32, tag="mmps")
                    for cch in range(2):
                        nc.tensor.matmul(
                            out=ps[:], lhsT=ws0[:, cch, o * P:(o + 1) * P],
                            rhs=h1T[:, cch, sl], start=(cch == 0), stop=False)
                        nc.tensor.matmul(
                            out=ps[:], lhsT=wa0[:, cch, o * P:(o + 1) * P],
                            rhs=n1T[:, cch, sl], start=False, stop=(cch == 1))
                    nc.scalar.activation(out=g1T[:, o, sl], in_=ps[:],
                                         func=AF.Relu)

            # --- nT = group-sums of g1T ---
            nf = vpool.tile([P, 2, EDGES], f32, tag="tr")
            tree10(g1T[:], nf[:], EDGES)
            nT = hpool.tile([P, 2, EDGES], f16, tag="nT")
            nc.vector.tensor_copy(nT[:], nf[:])

            # --- g0 = relu(h0 @ Ws0 + n0 @ Wa0) ---
            g0T = gpool.tile([P, 2, EDGES], f16, tag="g0T")
            for o in range(2):
                ps = ps2.tile([P, 512], f32, tag="mmps")
                for cch in range(2):
                    nc.tensor.matmul(out=ps[:],
                                     lhsT=ws0[:, cch, o * P:(o + 1) * P],
                                     rhs=h0T[:, cch, :], start=(cch == 0),
                                     stop=False)
                    nc.tensor.matmul(out=ps[:],
                                     lhsT=wa0[:, cch, o * P:(o + 1) * P],
                                     rhs=n0T[:, cch, :], start=False,
                                     stop=(cch == 1))
                nc.scalar.activation(out=g0T[:, o, :], in_=ps[:], func=AF.Relu)

            # --- hT = g0 @ Ws1 + n @ Wa1 ---
            ps = ps2.tile([P, 512], f32, tag="mmps")
            for cch in range(2):
                nc.tensor.matmul(out=ps[:], lhsT=ws1[:, cch, :],
                                 rhs=g0T[:, cch, :], start=(cch == 0),
                                 stop=False)
                nc.tensor.matmul(out=ps[:], lhsT=wa1[:, cch, :],
                                 rhs=nT[:, cch, :], start=False,
                                 stop=(cch == 1))
            hT = gpool.tile([P, EDGES], f16, tag=f"hT{sd}")
            nc.scalar.activation(out=hT[:], in_=ps[:], func=AF.Copy)
            hts[sd] = hT

        prod = gpool.tile([P, EDGES], f16, tag="prod")
        nc.vector.tensor_mul(prod[:], hts["s"][:], hts["d"][:])
        psf = ps2.tile([1, EDGES], f32, tag="mmps")
        nc.tensor.matmul(out=psf[:], lhsT=wlin_s[:], rhs=prod[:],
                         start=True, stop=True)
        res = gpool.tile([1, EDGES], f32, tag="res")
        nc.scalar.activation(out=res[:], in_=psf[:], func=AF.Identity,
                             bias=blin_s[:, :1])
        nc.sync.dma_start(out[:], res[:])

    nc.compile()
    return nc


def kernel(**inputs) -> np.ndarray:
    from concourse.bass_utils import run_bass_kernel_spmd

    plan = _build_plans(inputs)

    trace = bool(os.environ.get("GNN_TRACE"))
    if trace:
        import timing_shim
        timing_shim.install()

    nc = _build_bass(plan)

    f16 = np.float16
    # feature tables, feature-major fp16 [512, N]
    feat_u16 = np.ascontiguousarray(
        np.asarray(inputs["user_feat"], np.float32).T).astype(f16)
    feat_i16 = np.ascontiguousarray(
        np.asarray(inputs["item_feat"], np.float32).T).astype(f16)
    xts = _proj_host(feat_u16, feat_i16, plan)

    w_pu = np.ascontiguousarray(
        np.asarray(inputs["W_pu"], np.float32).reshape(4, P, D)
        .transpose(1, 0, 2)).astype(f16)
    w_pi = np.ascontiguousarray(
        np.asarray(inputs["W_pi"], np.float32).reshape(4, P, D)
        .transpose(1, 0, 2)).astype(f16)
    b_p = np.stack([np.tile(np.asarray(inputs["b_pu"], np.float32), 2),
                    np.tile(np.asarray(inputs["b_pi"], np.float32), 2)])[None].astype(f16)

    def sagew(pre):
        s0 = np.asarray(inputs[f"{pre}_self0"], np.float32)
        a0 = np.asarray(inputs[f"{pre}_agg0"], np.float32) * (1.0 / F0)
        s1 = np.asarray(inputs[f"{pre}_self1"], np.float32)
        a1 = np.asarray(inputs[f"{pre}_agg1"], np.float32) * (1.0 / F0)
        cat = np.concatenate([s0, a0, s1, a1], axis=1)  # [256, 768]
        return cat.reshape(2, P, 768).transpose(1, 0, 2)

    wsage = np.ascontiguousarray(
        np.concatenate([sagew("u"), sagew("i")], axis=2)).astype(f16)
    wlin = np.asarray(inputs["W_lin"], np.float32).astype(f16)
    blin = np.asarray(inputs["b_lin"], np.float32).reshape(1, 1)

    in_maps = []
    for c in range(NCORES):
        m = {
            "xt": xts[c], "w_pu": w_pu, "w_pi": w_pi, "b_p": b_p,
            "wsage": wsage, "wlin": wlin, "blin": blin,
            "idx_s": _wrap16(plan["idx2"][c]["s"]),
            "idx_d": _wrap16(plan["idx2"][c]["d"]),
        }
        in_maps.append(m)

    kw = dict(trace=True, trace_cores=list(range(NCORES))) if trace else {}
    res = run_bass_kernel_spmd(nc, in_maps, core_ids=list(range(NCORES)), **kw)
    if trace and res.exec_time_ns:
        print(f"HW exec time: {res.exec_time_ns} ns")
        kernel.last_exec_ns = res.exec_time_ns

    logits = np.concatenate([res.results[c]["out"][0] for c in range(NCORES)])
    return logits.reshape(B, 1).astype(np.float32)


# revision 19
# speedup vs baseline: 1.9827x; 1.0256x over previous
"""GraphSAGE (2-layer, mean-agg) edge-scoring kernel for 8 trn2 NeuronCores.

Per-core private compacted tables (no pair sharing, no barrier):
  - h0/h1 hop tables stored in token order (plain / transposing DMA loads).
  - hop-2 rows deduped into 2 tables per side (segs {0,1} and {2,3,4}),
    each guaranteed <= 30720 rows so plain positive int16 dma_gather idx work
    with no chunking and no reorder (gathers are order-preserving,
    1024-idx calls, k-major so the 10-way mean is 9 strided vector adds).
  - Projection reads host-prepacked fp16 feature tiles (HWDGE), writes each
    table slot contiguously; per-table dram tensors let Tile overlap SAGE
    gathers with the tail of projection.
  - SAGE matmuls run feat-major; token-major gathered data is transposed
    via TensorE (identity matmul) in packed PSUM groups.
"""
import os
import numpy as np

F0 = F1 = 10
B = 4096
NCORES = 8
EDGES = B // NCORES          # 512
P = 128
D = 256
NU, NI = 50000, 100000
SEG = 1024                   # hop-2 groups per segment
NSEG = (EDGES * F0) // SEG   # 5
NSEG_A = 2                   # segs 0,1 -> table A; segs 2.. -> table B
CALL = 1024                  # dma_gather idx per call (HW cap)
PROJ_TILE = 512


def _wrap16(a):
    a = np.asarray(a, np.int16)
    w = a.reshape(-1, 16).T
    return np.tile(w, (8, 1)).astype(np.int16)


def _pad512(n):
    return max(512, (int(n) + 511) & ~511)


def _build_plans(inputs):
    """Per-core compacted tables + k-major hop-2 gather indices.

    Returns dict with:
      sizes: dict slot -> padded row count (max over cores)
      rows:  per core: dict slot -> original-table row ids (np.int64 array)
      idx2:  per core: dict side -> [NSEG*F1*CALL] int16 gather idx (k-major)
    """
    h = {}
    for k in ("src_h0", "src_h1", "src_h2", "dst_h0", "dst_h1", "dst_h2"):
        h[k] = np.asarray(inputs[k]).astype(np.int64).reshape(NCORES, -1)

    rows = [dict() for _ in range(NCORES)]
    idx2 = [dict() for _ in range(NCORES)]
    for c in range(NCORES):
        for sd in ("s", "d"):
            pre = "src" if sd == "s" else "dst"
            h0 = h[pre + "_h0"][c]
            h1 = h[pre + "_h1"][c]
            h2 = h[pre + "_h2"][c]
            rows[c][sd + "h0"] = h0
            rows[c][sd + "h1"] = h1
            a_tok = h2[: NSEG_A * SEG * F1]
            b_tok = h2[NSEG_A * SEG * F1:]
            ua, inva = np.unique(a_tok, return_inverse=True)
            ub, invb = np.unique(b_tok, return_inverse=True)
            assert len(ua) <= 32000 and len(ub) <= 32000
            rows[c][sd + "A"] = ua
            rows[c][sd + "B"] = ub
            calls = []
            for s in range(NSEG):
                inv = inva if s < NSEG_A else invb
                off = 0 if s < NSEG_A else NSEG_A * SEG * F1
                base = s * SEG * F1 - off
                for k in range(F1):
                    # call position q (=token within seg) -> table pos
                    calls.append(inv[base + np.arange(SEG) * F1 + k])
            idx2[c][sd] = np.concatenate(calls)
    sizes = {}
    for slot in ("sh0", "sh1", "sA", "sB", "dh0", "dh1", "dA", "dB"):
        sizes[slot] = _pad512(max(len(rows[c][slot]) for c in range(NCORES)))
    return {"sizes": sizes, "rows": rows, "idx2": idx2}


# slot -> (feature kind, bias column) ; src side: h0/h2=user, h1=item
_SLOT_KIND = {"sh0": "u", "sh1": "i", "sA": "u", "sB": "u",
              "dh0": "i", "dh1": "u", "dA": "i", "dB": "i"}
# hop-2 seg-A tables first so their gathers overlap the rest of projection
_SLOTS = ("sA", "dA", "sh1", "dh1", "sh0", "dh0", "sB", "dB")


def _proj_host(feat_u16, feat_i16, plan):
    """Build per-core xt: [512, TOT] fp16, feature-major, tile-permuted.

    Within each 512-col tile, col q = cdim*128 + m holds row m*4 + cdim of
    the slot (so the projected PSUM groups store as contiguous row spans).
    """
    sizes = plan["sizes"]
    xts = []
    for c in range(NCORES):
        parts = []
        for slot in _SLOTS:
            r = plan["rows"][c][slot]
            n = sizes[slot]
            cols = np.zeros((n,), np.int64)
            cols[:len(r)] = r
            perm = cols.reshape(n // PROJ_TILE, P, 4).transpose(
                0, 2, 1).reshape(-1)
            feat = feat_u16 if _SLOT_KIND[slot] == "u" else feat_i16
            parts.append(feat[:, perm])
        xts.append(np.ascontiguousarray(np.concatenate(parts, axis=1)))
    return xts


def _build_bass(plan, debug=False):
    import concourse.tile as tile
    import concourse.bacc as bacc
    from concourse import mybir, library_config
    from concourse.masks import make_identity
    from contextlib import ExitStack

    f16 = mybir.dt.float16
    f32 = mybir.dt.float32
    i16 = mybir.dt.int16
    AF = mybir.ActivationFunctionType

    sizes = plan["sizes"]
    tot = sum(sizes[s] for s in _SLOTS)

    nc = bacc.Bacc("TRN2", target_bir_lowering=False, debug=False,
                   num_devices=NCORES, num_swdge_queues=4)

    xt = nc.dram_tensor("xt", [512, tot], f16, kind="ExternalInput")
    w_pu = nc.dram_tensor("w_pu", [P, 4, D], f16, kind="ExternalInput")
    w_pi = nc.dram_tensor("w_pi", [P, 4, D], f16, kind="ExternalInput")
    b_p = nc.dram_tensor("b_p", [1, 2, 2 * D], f16, kind="ExternalInput")
    wsage = nc.dram_tensor("wsage", [P, 2, 2 * 768], f16, kind="ExternalInput")
    wlin = nc.dram_tensor("wlin", [P, 1], f16, kind="ExternalInput")
    blin = nc.dram_tensor("blin", [1, 1], f32, kind="ExternalInput")
    idx_s = nc.dram_tensor("idx_s", [P, NSEG * F1 * CALL // 16], i16,
                           kind="ExternalInput")
    idx_d = nc.dram_tensor("idx_d", [P, NSEG * F1 * CALL // 16], i16,
                           kind="ExternalInput")
    out = nc.dram_tensor("out", [1, EDGES], f32, kind="ExternalOutput")

    tabs = {s: nc.dram_tensor(f"tab_{s}", [sizes[s], D], f16)
            for s in _SLOTS}

    with tile.TileContext(nc) as tc, ExitStack() as ctx:
        nc.gpsimd.load_library(library_config.mlp)
        tc.strict_bb_all_engine_barrier()

        wpool = ctx.enter_context(tc.tile_pool(name="w", bufs=1))
        w_pu_s = wpool.tile([P, 4, D], f16, tag="wpu")
        w_pi_s = wpool.tile([P, 4, D], f16, tag="wpi")
        b_p_s = wpool.tile([1, 2, 2 * D], f16, tag="bp")
        wsage_s = wpool.tile([P, 2, 2 * 768], f16, tag="wsage")
        wlin_s = wpool.tile([P, 1], f16, tag="wlin")
        blin_s = wpool.tile([1, 1], f32, tag="blin")
        ones_s = wpool.tile([1, P], f16, tag="ones")
        ident = wpool.tile([P, P], f16, tag="ident")
        for dst_, src_ in ((w_pu_s, w_pu), (w_pi_s, w_pi), (b_p_s, b_p),
                           (wsage_s, wsage), (wlin_s, wlin), (blin_s, blin)):
            nc.sync.dma_start(dst_[:], src_[:])
        nc.vector.memset(ones_s[:], 1.0)
        make_identity(nc, ident[:])

        # ---------------- phase A: projection ----------------
        ppool = ctx.enter_context(tc.tile_pool(name="proj", bufs=2))
        pspool = ctx.enter_context(tc.tile_pool(name="pps", bufs=2,
                                                space="PSUM"))
        col_off = 0
        for slot in _SLOTS:
            tab = tabs[slot]
            kind = _SLOT_KIND[slot]
            w_s = w_pu_s if kind == "u" else w_pi_s
            bcol = 0 if kind == "u" else 1
            ntiles = sizes[slot] // PROJ_TILE
            for t in range(ntiles):
                base = col_off + t * PROJ_TILE
                xtt = ppool.tile([P, 4, PROJ_TILE], f16, tag="xtt")
                nc.sync.dma_start(
                    xtt[:],
                    xt[:, base:base + PROJ_TILE].rearrange(
                        "(c p) n -> p c n", p=P))
                sig = ppool.tile([P, 4, D], f16, tag="sig")
                sgl = sig[:].rearrange("p a b -> p (a b)")
                # one PSUM bank per 2 j-groups; bias matmul opens the
                # accumulation group over the whole bank, mains accumulate
                for hb in range(2):
                    ps = pspool.tile([P, 2, D], f32, tag="pps")
                    psl = ps[:].rearrange("p a b -> p (a b)")
                    nc.tensor.matmul(out=psl[:, :], lhsT=ones_s[:, :],
                                     rhs=b_p_s[:, bcol, :], start=True,
                                     stop=False)
                    for jj in range(2):
                        j = 2 * hb + jj
                        for cch in range(4):
                            nc.tensor.matmul(
                                out=ps[:, jj, :],
                                lhsT=xtt[:, cch, j * P:(j + 1) * P],
                                rhs=w_s[:, cch, :], start=False,
                                stop=(cch == 3))
                    nc.scalar.activation(out=sgl[:, hb * 512:(hb + 1) * 512],
                                         in_=psl[:, :], func=AF.Sigmoid)
                nc.scalar.dma_start(
                    tab[t * PROJ_TILE:(t + 1) * PROJ_TILE, :].rearrange(
                        "(p r) d -> p (r d)", r=4),
                    sig[:].rearrange("p r d -> p (r d)"))
            col_off += sizes[slot]

        # ---------------- phase C: SAGE ----------------
        hpool = ctx.enter_context(tc.tile_pool(name="hts", bufs=1))
        # shared scratch: [P,8,256] f32 is the largest shape under this tag
        vpool = ctx.enter_context(tc.tile_pool(name="vtmp", bufs=3))
        plpool = ctx.enter_context(tc.tile_pool(name="pl", bufs=4))
        acpool = ctx.enter_context(tc.tile_pool(name="ac", bufs=2))
        i2pool = ctx.enter_context(tc.tile_pool(name="i2p", bufs=2))
        gpool = ctx.enter_context(tc.tile_pool(name="gts", bufs=1))
        ps2 = ctx.enter_context(tc.tile_pool(name="ps2", bufs=2, space="PSUM"))
        pst = ctx.enter_context(tc.tile_pool(name="pst", bufs=2, space="PSUM"))

        qn = [0]

        def tree10(src, dst, ngr):
            # src [P,2,ngr*10] f16 (col g*10+k) -> dst [P,2,ngr] f32 sums
            t0_t = vpool.tile([P, 2, EDGES], f32, tag="tr")
            t0 = t0_t[:, :, :ngr]
            t1_t = vpool.tile([P, 2, EDGES], f32, tag="tr")
            t1 = t1_t[:, :, :ngr]
            v = src.rearrange("p c (j k) -> p c j k", k=F0)
            nc.vector.tensor_add(t0[:], v[:, :, :, 0], v[:, :, :, 1])
            for i in range(1, 5):
                nc.vector.tensor_add(t1[:], v[:, :, :, 2 * i],
                                     v[:, :, :, 2 * i + 1])
                if i < 4:
                    nc.vector.tensor_add(t0[:], t0[:], t1[:])
            nc.vector.tensor_add(dst, t0[:], t1[:])

        hts = {}
        for si, sd in enumerate(("s", "d")):
            wof = si * 768
            ws0 = wsage_s[:, :, wof:wof + D]
            wa0 = wsage_s[:, :, wof + D:wof + 2 * D]
            ws1 = wsage_s[:, :, wof + 2 * D:wof + 2 * D + 128]
            wa1 = wsage_s[:, :, wof + 2 * D + 128:wof + 768]
            tab_h0 = tabs[sd + "h0"]
            tab_h1 = tabs[sd + "h1"]

            # --- h0T / h1T via transposing DMA loads (token order tables) ---
            h0T = hpool.tile([P, 2, EDGES], f16, tag="h0T")
            h1T = hpool.tile([P, 2, EDGES * F0], f16, tag="h1T")
            for f in range(2):
                nc.sync.dma_start_transpose(
                    h0T[:, f, :], tab_h0[:, f * P:(f + 1) * P])
                nc.sync.dma_start_transpose(
                    h1T[:, f, :], tab_h1[:, f * P:(f + 1) * P])

            # --- n0T = group-sums of h1T ---
            n0f = vpool.tile([P, 2, EDGES], f32, tag="tr")
            tree10(h1T[:], n0f[:], EDGES)
            n0T = hpool.tile([P, 2, EDGES], f16, tag="n0T")
            nc.vector.tensor_copy(n0T[:], n0f[:])

            # --- hop-2 segments -> n1T (rotating k-planes, chained accum) ---
            i2t = i2pool.tile([P, NSEG * F1 * CALL // 16], i16, tag="i2t")
            nc.sync.dma_start(i2t[:], (idx_s if sd == "s" else idx_d)[:])
            n1T = hpool.tile([P, 2, EDGES * F0], f16, tag="n1T")
            for s in range(NSEG):
                tabAB = tabs[sd + ("A" if s < NSEG_A else "B")]
                acc = acpool.tile([P, SEG // P, D], f16, tag="acc")
                planes = []
                for k in range(F1):
                    pl = plpool.tile([P, SEG // P, D], f16, tag="pl")
                    co = (s * F1 + k) * (CALL // 16)
                    nc.gpsimd.dma_gather(
                        pl[:], tabAB[:, :], i2t[:, co:co + CALL // 16],
                        CALL, CALL, D, queue_num=qn[0] % 4)
                    qn[0] += 1
                    planes.append(pl)
                    if k == 1:
                        nc.vector.tensor_add(acc[:], planes[0][:], planes[1][:])
                    elif k > 1:
                        nc.vector.tensor_add(acc[:], acc[:], pl[:])
                n1s = acc
                # transpose to feat-major: 16 tiles packed 4-per-psum
                for f in range(2):
                    for g in range(2):
                        pt = pst.tile([P, 4, P], f16, tag="pt")
                        for b in range(4):
                            nc.tensor.matmul(
                                out=pt[:, b, :],
                                lhsT=n1s[:, g * 4 + b, f * P:(f + 1) * P],
                                rhs=ident[:], is_transpose=True)
                        dst = n1T[:, f, s * SEG + g * 512:
                                  s * SEG + (g + 1) * 512]
                        if (f + g) % 2 == 0:
                            nc.scalar.activation(out=dst, in_=pt[:].rearrange(
                                "p b q -> p (b q)"), func=AF.Copy)
                        else:
                            nc.vector.tensor_copy(dst, pt[:].rearrange(
                                "p b q -> p (b q)"))

            # --- g1 = relu(h1 @ Ws0 + n1 @ Wa0) ---
            g1T = gpool.tile([P, 2, EDGES * F0], f16, tag="g1T")
            for o in range(2):
                for bb in range(EDGES * F0 // 512):
                    sl = slice(bb * 512, (bb + 1) * 512)
                    ps = ps2.tile([P, 512], f32, tag="mmps")
                    for cch in range(2):
                        nc.tensor.matmul(
                            out=ps[:], lhsT=ws0[:, cch, o * P:(o + 1) * P],
                            rhs=h1T[:, cch, sl], start=(cch == 0), stop=False)
                        nc.tensor.matmul(
                            out=ps[:], lhsT=wa0[:, cch, o * P:(o + 1) * P],
                            rhs=n1T[:, cch, sl], start=False, stop=(cch == 1))
                    nc.scalar.activation(out=g1T[:, o, sl], in_=ps[:],
                                         func=AF.Relu)

            # --- nT = group-sums of g1T ---
            nf = vpool.tile([P, 2, EDGES], f32, tag="tr")
            tree10(g1T[:], nf[:], EDGES)
            nT = hpool.tile([P, 2, EDGES], f16, tag="nT")
            nc.vector.tensor_copy(nT[:], nf[:])

            # --- g0 = relu(h0 @ Ws0 + n0 @ Wa0) ---
            g0T = gpool.tile([P, 2, EDGES], f16, tag="g0T")
            for o in range(2):
                ps = ps2.tile([P, 512], f32, tag="mmps")
                for cch in range(2):
                    nc.tensor.matmul(out=ps[:],
                                     lhsT=ws0[:, cch, o * P:(o + 1) * P],
                                     rhs=h0T[:, cch, :], start=(cch == 0),
                                     stop=False)
                    nc.tensor.matmul(out=ps[:],
                                     lhsT=wa0[:, cch, o * P:(o + 1) * P],
                                     rhs=n0T[:, cch, :], start=False,
                                     stop=(cch == 1))
                nc.scalar.activation(out=g0T[:, o, :], in_=ps[:], func=AF.Relu)

            # --- hT = g0 @ Ws1 + n @ Wa1 ---
            ps = ps2.tile([P, 512], f32, tag="mmps")
            for cch in range(2):
                nc.tensor.matmul(out=ps[:], lhsT=ws1[:, cch, :],
                                 rhs=g0T[:, cch, :], start=(cch == 0),
                                 stop=False)
                nc.tensor.matmul(out=ps[:], lhsT=wa1[:, cch, :],
                                 rhs=nT[:, cch, :], start=False,
                                 stop=(cch == 1))
            hT = gpool.tile([P, EDGES], f16, tag=f"hT{sd}")
            nc.scalar.activation(out=hT[:], in_=ps[:], func=AF.Copy)
            hts[sd] = hT

        prod = gpool.tile([P, EDGES], f16, tag="prod")
        nc.vector.tensor_mul(prod[:], hts["s"][:], hts["d"][:])
        psf = ps2.tile([1, EDGES], f32, tag="mmps")
        nc.tensor.matmul(out=psf[:], lhsT=wlin_s[:], rhs=prod[:],
                         start=True, stop=True)
        res = gpool.tile([1, EDGES], f32, tag="res")
        nc.scalar.activation(out=res[:], in_=psf[:], func=AF.Identity,
                             bias=blin_s[:, :1])
        nc.sync.dma_start(out[:], res[:])

    nc.compile()
    return nc


def kernel(**inputs) -> np.ndarray:
    from concourse.bass_utils import run_bass_kernel_spmd

    plan = _build_plans(inputs)

    trace = bool(os.environ.get("GNN_TRACE"))
    if trace:
        import timing_shim
        timing_shim.install()

    nc = _build_bass(plan)

    f16 = np.float16
    # feature tables, feature-major fp16 [512, N]
    feat_u16 = np.ascontiguousarray(
        np.asarray(inputs["user_feat"], np.float32).T).astype(f16)
    feat_i16 = np.ascontiguousarray(
        np.asarray(inputs["item_feat"], np.float32).T).astype(f16)
    xts = _proj_host(feat_u16, feat_i16, plan)

    w_pu = np.ascontiguousarray(
        np.asarray(inputs["W_pu"], np.float32).reshape(4, P, D)
        .transpose(1, 0, 2)).astype(f16)
    w_pi = np.ascontiguousarray(
        np.asarray(inputs["W_pi"], np.float32).reshape(4, P, D)
        .transpose(1, 0, 2)).astype(f16)
    b_p = np.stack([np.tile(np.asarray(inputs["b_pu"], np.float32), 2),
                    np.tile(np.asarray(inputs["b_pi"], np.float32), 2)])[None].astype(f16)

    def sagew(pre):
        s0 = np.asarray(inputs[f"{pre}_self0"], np.float32)
        a0 = np.asarray(inputs[f"{pre}_agg0"], np.float32) * (1.0 / F0)
        s1 = np.asarray(inputs[f"{pre}_self1"], np.float32)
        a1 = np.asarray(inputs[f"{pre}_agg1"], np.float32) * (1.0 / F0)
        cat = np.concatenate([s0, a0, s1, a1], axis=1)  # [256, 768]
        return cat.reshape(2, P, 768).transpose(1, 0, 2)

    wsage = np.ascontiguousarray(
        np.concatenate([sagew("u"), sagew("i")], axis=2)).astype(f16)
    wlin = np.asarray(inputs["W_lin"], np.float32).astype(f16)
    blin = np.asarray(inputs["b_lin"], np.float32).reshape(1, 1)

    in_maps = []
    for c in range(NCORES):
        m = {
            "xt": xts[c], "w_pu": w_pu, "w_pi": w_pi, "b_p": b_p,
            "wsage": wsage, "wlin": wlin, "blin": blin,
            "idx_s": _wrap16(plan["idx2"][c]["s"]),
            "idx_d": _wrap16(plan["idx2"][c]["d"]),
        }
        in_maps.append(m)

    kw = dict(trace=True, trace_cores=list(range(NCORES))) if trace else {}
    res = run_bass_kernel_spmd(nc, in_maps, core_ids=list(range(NCORES)), **kw)
    if trace and res.exec_time_ns:
        print(f"HW exec time: {res.exec_time_ns} ns")
        kernel.last_exec_ns = res.exec_time_ns

    logits = np.concatenate([res.results[c]["out"][0] for c in range(NCORES)])
    return logits.reshape(B, 1).astype(np.float32)
